# revision 1
# baseline (speedup 1.0000x reference)
"""Trainium2 Bass kernel for a decoder block (LN->attn->residual->LN->FFN->residual).

Sharding: 8 NeuronCores. Cores 0-3 handle batch 0, cores 4-7 batch 1.
Core 4*b + p owns the contiguous 512-token block p of its batch.
q (feature-major) and v (row-major, with a per-head ones column that makes the
softmax denominator fall out of the attention matmul) are AllGathered within
each 4-core batch group. Attention runs dense (all 16 j-chunks) with causal
masking applied as data (per-core mask tensors), which keeps the SPMD program
identical on every core.

The reference computes scores[i,j] = k_i . q_j, softmaxes over j and applies a
j<=i causal mask, so k plays the usual "query" role and q/v the context role.
Logits are bounded (|s|*scale < ~10), so softmax is computed as exp(s)/sum
without max subtraction.

Shapes (hardcoded): B=2, T=2048, C=1024, H=16 heads, D=64, F=4096.
"""

import sys

sys.path.insert(0, "/opt/trn_rl_repo")

import numpy as np
import ml_dtypes

import concourse.bass as bass
import concourse.bacc as bacc
import concourse.tile as tile
from concourse import mybir
from concourse.bass_utils import run_bass_kernel_spmd
from concourse.masks import make_identity

F32 = mybir.dt.float32
BF16 = mybir.dt.bfloat16
AF = mybir.ActivationFunctionType
OP = mybir.AluOpType

B, T, C = 2, 2048, 1024
H, D = 16, 64
F = 4 * C
EPS = 1e-3
N_CORES = 8
GROUPS = [[0, 1, 2, 3], [4, 5, 6, 7]]
RT = 512          # tokens per core
NT = RT // 128    # 4 local token chunks of 128
NJ = 16           # j-chunks of 128 across the batch's full T
NC_F = C // 128   # 8 feature chunks
NF_F = F // 128   # 32 ffn feature chunks
SCALE = 1.0 / float(np.sqrt(D))

# AllGather payload per rank: qT [C, RT] + v_aug [RT, H*65], bf16
QT_ELEMS = C * RT
VA_ELEMS = RT * H * 65
AG_ELEMS = QT_ELEMS + VA_ELEMS


def build_nc(reps=1, affine=True):
    nc = bacc.Bacc(None, target_bir_lowering=False)

    x_d = nc.dram_tensor("x_loc", [RT, C], F32, kind="ExternalInput")
    wq_d = nc.dram_tensor("Wq", [C, C], BF16, kind="ExternalInput")
    wk_d = nc.dram_tensor("Wk", [C, C], BF16, kind="ExternalInput")
    wv_d = nc.dram_tensor("Wv", [C, C], BF16, kind="ExternalInput")
    w1_d = nc.dram_tensor("W1", [C, F], BF16, kind="ExternalInput")
    w2_d = nc.dram_tensor("W2", [F, C], BF16, kind="ExternalInput")
    bq_d = nc.dram_tensor("bq", [C], F32, kind="ExternalInput")
    bk_d = nc.dram_tensor("bk", [C], F32, kind="ExternalInput")
    bv_d = nc.dram_tensor("bv", [C], F32, kind="ExternalInput")
    b1_d = nc.dram_tensor("b1", [F], F32, kind="ExternalInput")
    b2_d = nc.dram_tensor("b2", [C], F32, kind="ExternalInput")
    g1_d = nc.dram_tensor("ln1_g", [C], F32, kind="ExternalInput")
    be1_d = nc.dram_tensor("ln1_b", [C], F32, kind="ExternalInput")
    g2_d = nc.dram_tensor("ln2_g", [C], F32, kind="ExternalInput")
    be2_d = nc.dram_tensor("ln2_b", [C], F32, kind="ExternalInput")
    mask_d = nc.dram_tensor("masks", [NJ, 128, RT], BF16, kind="ExternalInput")
    y_d = nc.dram_tensor("y", [RT, C], F32, kind="ExternalOutput")

    agq_in = nc.dram_tensor("agq_in", [QT_ELEMS], BF16)
    agq_out = nc.dram_tensor("agq_out", [4 * QT_ELEMS], BF16)
    agv_in = nc.dram_tensor("agv_in", [VA_ELEMS], BF16)
    agv_out = nc.dram_tensor("agv_out", [4 * VA_ELEMS], BF16)

    def bcast_row(dram_vec):
        # [n] dram vector -> [128, n] broadcast AP (partition step 0)
        return bass.AP(tensor=dram_vec.tensor, offset=dram_vec.offset,
                       ap=[[0, 128], dram_vec.ap[0]])

    with tile.TileContext(nc) as tc:
        with (
            tc.tile_pool(name="const", bufs=1) as const,
            tc.tile_pool(name="big", bufs=1) as big,
            tc.tile_pool(name="wpool", bufs=1) as wpool,
            tc.tile_pool(name="w1s", bufs=3) as w1s,
            tc.tile_pool(name="stats", bufs=4) as stats,
            tc.tile_pool(name="exs", bufs=4) as exs,
            tc.tile_pool(name="avs_pool", bufs=2) as avs_pool,
            tc.tile_pool(name="qs_pool", bufs=2) as qs_pool,
            tc.tile_pool(name="vs_pool", bufs=2) as vs_pool,
            tc.tile_pool(name="mm_ps", bufs=2, space="PSUM") as mm_ps,
            tc.tile_pool(name="st_ps", bufs=2, space="PSUM") as st_ps,
            tc.tile_pool(name="av_ps", bufs=2, space="PSUM") as av_ps,
        ):
            # ---- load x first (per chunk) so LN1 starts immediately ----
            x_first = big.tile([128, NT, C], F32, tag="x", name="x_first")
            for t in range(NT):
                nc.sync.dma_start(out=x_first[:, t],
                                  in_=x_d.rearrange("(t p) c -> p t c", p=128)[:, t])

            # ---- constants ----
            ident_b = const.tile([128, 128], BF16, tag="ident_b")
            make_identity(nc, ident_b)
            ident_f = const.tile([128, 128], F32, tag="ident_f")
            make_identity(nc, ident_f)
            eps_t = const.tile([128, 1], F32, tag="eps")
            nc.vector.memset(eps_t, EPS)
            # warm the ACT function tables (Sqrt/Exp/Gelu) while x streams in
            warm = stats.tile([128, 1], F32, tag="rs")
            nc.scalar.activation(out=warm, in_=eps_t, func=AF.Sqrt)
            nc.scalar.activation(out=warm, in_=eps_t, func=AF.Exp)
            nc.scalar.activation(out=warm, in_=eps_t, func=AF.Gelu)

            g1_r = const.tile([128, 2, C], F32, tag="lnr")
            nc.sync.dma_start(out=g1_r[:, 0], in_=bcast_row(g1_d[:]))
            nc.sync.dma_start(out=g1_r[:, 1], in_=bcast_row(be1_d[:]))
            bv_r = const.tile([128, C], F32, tag="rowr")
            nc.sync.dma_start(out=bv_r, in_=bcast_row(bv_d[:]))

            bq_t = const.tile([128, NC_F], F32, tag="bq")
            nc.sync.dma_start(out=bq_t, in_=bq_d.rearrange("(a p) -> p a", p=128))
            bk_t = const.tile([128, NC_F], F32, tag="bk")
            nc.sync.dma_start(out=bk_t, in_=bk_d.rearrange("(a p) -> p a", p=128))
            b1_t = const.tile([128, NF_F], F32, tag="b1")
            nc.sync.dma_start(out=b1_t, in_=b1_d.rearrange("(a p) -> p a", p=128))

            # ---- body (repeated `reps` times for benchmarking) ----
            for _rep in range(reps):
              # ---- x (preloaded before constants for rep 0) ----
              if _rep == 0:
                  x_sb = x_first
              else:
                  x_sb = big.tile([128, NT, C], F32, tag="x")
                  nc.sync.dma_start(out=x_sb, in_=x_d.rearrange("(t p) c -> p t c", p=128))

              def layernorm(src_tile, h_out, g_pair):
                  # src [128, C] f32 -> h_out [128, C] bf16; g_pair [128, 2, C] f32
                  st6 = stats.tile([128, 2, 6], F32, tag="bnst")
                  for s in range(2):
                      nc.vector.bn_stats(out=st6[:, s], in_=src_tile[:, 512 * s:512 * (s + 1)])
                  mv = stats.tile([128, 2], F32, tag="bnmv")
                  nc.vector.bn_aggr(out=mv, in_=st6)
                  rs = stats.tile([128, 1], F32, tag="rs")
                  nc.scalar.activation(out=rs, in_=mv[:, 1:2], func=AF.Sqrt, bias=eps_t)
                  nc.vector.reciprocal(out=rs, in_=rs)
                  nc.vector.tensor_scalar(out=h_out, in0=src_tile, scalar1=mv[:, 0:1],
                                          scalar2=rs, op0=OP.subtract, op1=OP.mult)
                  if affine:
                      nc.vector.tensor_mul(out=h_out, in0=h_out, in1=g_pair[:, 0])
                      nc.vector.tensor_add(out=h_out, in0=h_out, in1=g_pair[:, 1])

              # ---- LN1 -> h1 bf16, transpose -> h1T ----
              h1 = big.tile([128, NT, C], BF16, tag="h_row")
              for t in range(NT):
                  layernorm(x_sb[:, t], h1[:, t], g1_r)
              h1T = big.tile([128, NC_F, RT], BF16, tag="hT")
              for t in range(NT):
                  for fc in range(NC_F):
                      pt = mm_ps.tile([128, 128], BF16, tag="mm")
                      nc.tensor.transpose(pt, h1[:, t, 128 * fc:128 * (fc + 1)], ident_b)
                      nc.vector.tensor_copy(out=h1T[:, fc, 128 * t:128 * (t + 1)], in_=pt)

              # ---- QKV (weights streamed); q first so its AllGather can
              # run while k/v are still being computed ----
              qT = big.tile([128, NC_F, RT], BF16, tag="A32")
              kT = big.tile([128, NC_F, RT], BF16, tag="kT")

              def proj(w_d_, b_t, outT):
                  w_view = w_d_.rearrange("(a p) c -> p a c", p=128)
                  for co in range(NC_F):
                      w_t = w1s.tile([128, NC_F, 128], BF16, tag="w1t")
                      nc.sync.dma_start(out=w_t, in_=w_view[:, :, 128 * co:128 * (co + 1)])
                      ps = mm_ps.tile([128, RT], F32, tag="mm")
                      for ci in range(NC_F):
                          nc.tensor.matmul(ps, w_t[:, ci, :],
                                           h1T[:, ci, :], start=(ci == 0), stop=(ci == NC_F - 1))
                      nc.scalar.activation(out=outT[:, co, :], in_=ps, func=AF.Identity,
                                           bias=b_t[:, co:co + 1])

              proj(wq_d, bq_t, qT)
              agq_view = agq_in[:].rearrange("(f p t) -> p f t", p=128, t=RT)
              for co in range(NC_F):
                  nc.sync.dma_start(out=agq_view[:, co], in_=qT[:, co])
              nc.gpsimd.collective_compute(
                  "AllGather", OP.bypass, replica_groups=GROUPS,
                  ins=[agq_in[:]], outs=[agq_out[:]])
              proj(wk_d, bk_t, kT)

              # v row-major with interleaved per-head ones column: [tok%128, tchunk, H, 65]
              v_aug = big.tile([128, NT, H, 65], BF16, tag="v_aug")
              nc.vector.memset(v_aug[:, :, :, 64:65], 1.0)
              wv_view = wv_d.rearrange("(a p) c -> p a c", p=128)
              for half in range(2):
                  wv_t = wpool.tile([128, NC_F, RT], BF16, tag="wvh", bufs=2)
                  nc.sync.dma_start(out=wv_t, in_=wv_view[:, :, 512 * half:512 * (half + 1)])
                  for t in range(NT):
                      ps = mm_ps.tile([128, RT], F32, tag="mm")
                      for ci in range(NC_F):
                          nc.tensor.matmul(ps, h1T[:, ci, 128 * t:128 * (t + 1)],
                                           wv_t[:, ci, :],
                                           start=(ci == 0), stop=(ci == NC_F - 1))
                      if affine:
                          nc.vector.tensor_tensor(
                              out=v_aug[:, t, 8 * half:8 * (half + 1), 0:64],
                              in0=ps.rearrange("p (a b) -> p a b", b=64),
                              in1=bv_r[:, 512 * half:512 * (half + 1)].rearrange(
                                  "p (a b) -> p a b", b=64),
                              op=OP.add)
                      else:
                          nc.vector.tensor_copy(
                              out=v_aug[:, t, 8 * half:8 * (half + 1), 0:64],
                              in_=ps.rearrange("p (a b) -> p a b", b=64))
              # ---- AllGather v within the 4-core batch group ----
              agv_view = agv_in[:].rearrange("(tc p x) -> p tc x", p=128, x=H * 65)
              for t in range(NT):
                  nc.sync.dma_start(out=agv_view[:, t],
                                    in_=v_aug[:, t].rearrange("p b c -> p (b c)"))
              nc.gpsimd.collective_compute(
                  "AllGather", OP.bypass, replica_groups=GROUPS,
                  ins=[agv_in[:]], outs=[agv_out[:]])

              # loads that overlap the collective: causal masks + first FFN
              # weight tiles (consumed after attention)
              mask_sb = big.tile([128, NJ, RT], BF16, tag="masks")
              nc.sync.dma_start(out=mask_sb, in_=mask_d.rearrange("m p i -> p m i"))
              w1_view = w1_d.rearrange("(a p) f -> p a f", p=128)
              w1_pre = []
              for fo in range(2):
                  w1_t = w1s.tile([128, NC_F, 128], BF16, tag="w1t",
                                  name=f"w1pre_{fo}")
                  nc.sync.dma_start(out=w1_t,
                                    in_=w1_view[:, :, 128 * fo:128 * (fo + 1)])
                  w1_pre.append(w1_t)


              # ---- attention (dense over NJ j-chunks; causality via mask data) ----
              # gathered q/v are streamed from DRAM per head-pair (each byte is
              # consumed by exactly one head).
              attn_sb = big.tile([128, NT, C], F32, tag="attn_out")
              q_src = agq_out.rearrange(
                  "(r f p t) -> p r f t", p=128, t=RT, r=4)     # [128, 4, NC_F, RT]
              v_src = agv_out.rearrange(
                  "(r tc p x) -> p r tc x", p=128, x=H * 65, r=4)  # [128, 4, NT, H*65]

              for a in range(H // 2):
                  q_pair = qs_pool.tile([128, 4, RT], BF16, tag="qpair")
                  nc.sync.dma_start(out=q_pair, in_=q_src[:, :, a, :])
                  v_pair = vs_pool.tile([128, 4, NT, 130], BF16, tag="vpair")
                  for r in range(4):
                      nc.sync.dma_start(out=v_pair[:, r],
                                        in_=v_src[:, r, :, 130 * a:130 * (a + 1)])

                  def q_slice(hi, m):
                      return q_pair[64 * hi:64 * hi + 64, m // 4,
                                    128 * (m % 4):128 * (m % 4) + 128]

                  def v_slice(hi, m):
                      return v_pair[:, m // 4, m % 4, 65 * hi:65 * hi + 65]

                  # both heads of the pair processed together: the two score
                  # matmuls use disjoint PE row groups (partitions 0:64/64:128)
                  # and run concurrently via tile_position row tiling.
                  avs2 = [av_ps.tile([65, RT], F32, tag="av", name=f"av_{a}_{k2}")
                           for k2 in range(2)]
                  for m in range(NJ):
                      st = st_ps.tile([128, 2, RT], F32, tag="st")
                      ex = exs.tile([128, 2, RT], BF16, tag="ex")
                      for hi in range(2):
                          nc.tensor.matmul(st[:, hi], q_slice(hi, m),
                                           kT[64 * hi:64 * hi + 64, a, :],
                                           start=True, stop=True,
                                           tile_position=(64 * hi, 0))
                      nc.scalar.activation(out=ex, in_=st, func=AF.Exp, scale=SCALE)
                      # one paired mask multiply for both heads (mask broadcast
                      # over the head dim via a 0-stride AP)
                      msl = mask_sb[:, m, :]
                      mask_b = bass.AP(tensor=msl.tensor, offset=msl.offset,
                                       ap=[msl.ap[0], [0, 2]] + list(msl.ap[1:]))
                      nc.vector.tensor_mul(out=ex, in0=ex, in1=mask_b)
                      for hi in range(2):
                          nc.tensor.matmul(avs2[hi], v_slice(hi, m), ex[:, hi],
                                           start=(m == 0), stop=(m == NJ - 1))

                  for hi in range(2):
                      h = 2 * a + hi
                      avs = avs_pool.tile([65, RT], F32, tag="avs")
                      nc.vector.tensor_copy(out=avs, in_=avs2[hi])
                      for i4 in range(NT):
                          pt = mm_ps.tile([128, 128], F32, tag="mm")
                          nc.tensor.transpose(pt[:, 0:65], avs[:, 128 * i4:128 * (i4 + 1)],
                                              ident_f[0:65, 0:65])
                          rec = stats.tile([128, 1], F32, tag="rec")
                          nc.vector.reciprocal(out=rec, in_=pt[:, 64:65])
                          nc.vector.tensor_scalar_mul(
                              out=attn_sb[:, i4, 64 * h:64 * (h + 1)],
                              in0=pt[:, 0:64], scalar1=rec)

              # ---- residual + LN2 -> h2, transpose -> h2T ----
              g2_r = const.tile([128, 2, C], F32, tag="lnr")
              nc.sync.dma_start(out=g2_r[:, 0], in_=bcast_row(g2_d[:]))
              nc.sync.dma_start(out=g2_r[:, 1], in_=bcast_row(be2_d[:]))
              h2 = big.tile([128, NT, C], BF16, tag="h_row")
              h2T = big.tile([128, NC_F, RT], BF16, tag="hT")
              for t in range(NT):
                  nc.vector.tensor_add(out=x_sb[:, t], in0=x_sb[:, t], in1=attn_sb[:, t])
                  layernorm(x_sb[:, t], h2[:, t], g2_r)
                  for fc in range(NC_F):
                      pt = mm_ps.tile([128, 128], BF16, tag="mm")
                      nc.tensor.transpose(pt, h2[:, t, 128 * fc:128 * (fc + 1)], ident_b)
                      nc.vector.tensor_copy(out=h2T[:, fc, 128 * t:128 * (t + 1)], in_=pt)

              # ---- FFN1 + gelu -> g1T ----
              g1T = big.tile([128, NF_F, RT], BF16, tag="A32")
              for fo in range(NF_F):
                  if fo < 2:
                      w1_t = w1_pre[fo]
                  else:
                      w1_t = w1s.tile([128, NC_F, 128], BF16, tag="w1t")
                      nc.sync.dma_start(out=w1_t,
                                        in_=w1_view[:, :, 128 * fo:128 * (fo + 1)])
                  ps = mm_ps.tile([128, RT], F32, tag="mm")
                  for ci in range(NC_F):
                      nc.tensor.matmul(ps, w1_t[:, ci, :], h2T[:, ci, :],
                                       start=(ci == 0), stop=(ci == NC_F - 1))
                  nc.scalar.activation(out=g1T[:, fo, :], in_=ps, func=AF.Gelu,
                                       bias=b1_t[:, fo:fo + 1])

              # ---- FFN2 + residual -> y (W2 streamed in quarters, double-buffered) ----
              b2_r = const.tile([128, C], F32, tag="rowr")
              nc.sync.dma_start(out=b2_r, in_=bcast_row(b2_d[:]))
              out_sb = big.tile([128, NT, C], F32, tag="attn_out")
              w2_view = w2_d.rearrange("(a p) c -> p a c", p=128)
              y_view = y_d.rearrange("(t p) c -> p t c", p=128)
              for q4 in range(4):
                  w2_sb = big.tile([128, NF_F, 256], BF16, tag="w2q", bufs=2)
                  nc.sync.dma_start(out=w2_sb,
                                    in_=w2_view[:, :, 256 * q4:256 * (q4 + 1)])
                  for t in range(NT):
                      ps = mm_ps.tile([128, 256], F32, tag="mm")
                      for fo in range(NF_F):
                          nc.tensor.matmul(ps, g1T[:, fo, 128 * t:128 * (t + 1)],
                                           w2_sb[:, fo, :],
                                           start=(fo == 0), stop=(fo == NF_F - 1))
                      dst = out_sb[:, t, 256 * q4:256 * (q4 + 1)]
                      nc.vector.tensor_tensor(out=dst, in0=ps,
                                              in1=x_sb[:, t, 256 * q4:256 * (q4 + 1)],
                                              op=OP.add)
                      if affine:
                          nc.vector.tensor_add(out=dst, in0=dst,
                                               in1=b2_r[:, 256 * q4:256 * (q4 + 1)])
                      if q4 == 3:
                          nc.sync.dma_start(out=y_view[:, t], in_=out_sb[:, t])

    nc.compile()
    return nc


_NC_CACHE = {}


def _get_nc(affine=True):
    if affine not in _NC_CACHE:
        _NC_CACHE[affine] = build_nc(affine=affine)
    return _NC_CACHE[affine]


def _affine_trivial(inputs):
    one = lambda a: np.allclose(np.asarray(a, np.float32), 1.0)
    zero = lambda a: not np.any(np.asarray(a, np.float32))
    return (one(inputs["ln1_g"]) and zero(inputs["ln1_b"])
            and one(inputs["ln2_g"]) and zero(inputs["ln2_b"])
            and zero(inputs["bv"]) and zero(inputs["b2"]))


def _make_masks(p_block):
    """Causal masks for the core owning rows [512p, 512p+512): [NJ, 128, RT]."""
    jj = 128 * np.arange(NJ)[:, None, None] + np.arange(128)[None, :, None]
    ii = 512 * p_block + np.arange(RT)[None, None, :]
    return (jj <= ii).astype(ml_dtypes.bfloat16)


def _prep_in_maps(inputs):
    x = np.asarray(inputs["x"], np.float32)
    cast_b = lambda a: np.asarray(np.asarray(a, np.float32)).astype(ml_dtypes.bfloat16)
    cast_f = lambda a: np.ascontiguousarray(np.asarray(a, np.float32))
    common = {
        "Wq": cast_b(inputs["Wq"]), "Wk": cast_b(inputs["Wk"]),
        "Wv": cast_b(inputs["Wv"]), "W1": cast_b(inputs["W1"]),
        "W2": cast_b(inputs["W2"]),
        "bq": cast_f(inputs["bq"]), "bk": cast_f(inputs["bk"]),
        "bv": cast_f(inputs["bv"]), "b1": cast_f(inputs["b1"]),
        "b2": cast_f(inputs["b2"]),
        "ln1_g": cast_f(inputs["ln1_g"]), "ln1_b": cast_f(inputs["ln1_b"]),
        "ln2_g": cast_f(inputs["ln2_g"]), "ln2_b": cast_f(inputs["ln2_b"]),
    }
    in_maps = []
    for core in range(N_CORES):
        b, p = core // 4, core % 4
        m = dict(common)
        m["x_loc"] = np.ascontiguousarray(x[b, 512 * p:512 * (p + 1)])
        m["masks"] = _make_masks(p)
        in_maps.append(m)
    return in_maps


def _assemble(results):
    out = np.empty((B, T, C), np.float32)
    for core in range(N_CORES):
        b, p = core // 4, core % 4
        out[b, 512 * p:512 * (p + 1)] = results[core]["y"]
    return out


def run_spmd(inputs, **kw):
    """Run on hardware; returns (full_output, BassKernelResults)."""
    in_maps = _prep_in_maps(inputs)
    nc = _get_nc(affine=not _affine_trivial(inputs))
    res = run_bass_kernel_spmd(nc, in_maps, core_ids=list(range(N_CORES)), **kw)
    return _assemble(res.results), res


def kernel(**inputs):
    out, _ = run_spmd(inputs)
    return out

